# revision 6
# baseline (speedup 1.0000x reference)
"""LSH cosine-of-Hamming retrieval kernel for 8 trn2 NeuronCores.

Math: reference computes cos((pi/d) * hamming(u, v)) for binary LSH codes
u = (emb1 @ r.T > 0), v = (emb2 @ r.T > 0), d = 1024 bits.
With +/-1 sign codes s_u = 2u-1, s_v = 2v-1:
    hamming = (d - s_u . s_v) / 2
    cos((pi/d) * hamming) = sin((pi/2d) * s_u.s_v)
u codes are stored as +/-1 fp8 (ACT Sign), v codes as +/-0.5 fp8 (DVE
is_gt/subtract), so psum P = 0.5 * s_u.s_v and out = sin((pi/d) * P).

Projection runs in ONE fp32r pass (the PE truncates operands to ~FP22
and products to similar precision; inputs are pre-rounded to 13
mantissa bits on the host so the input truncation is a no-op, leaving
~6e-5 sign-flip rate -> 9.6e-3 rel err, inside the 2e-2 budget). This
replaces the 3-pass bf16 hi/lo split projection: 64 instead of 192
projection matmuls (fp32r streams ~1.7 cycles/row vs bf16's 1.0, so
projection PE time drops 41us -> ~24us).

The binarize of the projection psum (4M fp32 values) is the side
constraint: GPSIMD cannot read PSUM and DVE 2x perf modes need 16-bit
or SBUF sources, so only DVE (~0.69us/bank) and ACT (~0.69us/bank) can
drain codes. Code-magnitude uniformity within each tensor forces u
entirely on ACT (Sign, +/-1) and v entirely on DVE (+/-0.5); both run
saturated through the projection phase and their tails overlap the
main-matmul phase (early halves only need v j0/j1 + u j0 codes).

Main phase: fp8 DoubleRow half-blocks (128 rows x 1024 cols), one
2-bank psum pair per half, Sin on ACT, output DMAs alternating
sync/gpsimd queues. The last two halves use finer Sin/DMA quanta to
shorten the drain.

NOTE: do NOT dedupe back-to-back identical LDWEIGHTS here - on this
runtime the PE's reorder window executes pulled-ahead weight loads
while early matmuls are parked on unsatisfied code semaphores, and the
second matmul of a pair then runs with clobbered weights (reproducible
NaN quadrants in the first scheduled halves).

Sharding (2x4 grid over 8 cores): core k computes the [2048, 2048]
output block for emb1 rows [(k//4)*2048...] x emb2 rows [(k%4)*2048...];
r is replicated (collectives cost ~60us fixed here - not worth it).
Host prep is layout-only: transpose + fp22 pre-round.
"""

import sys

sys.path.insert(0, "/opt/trn_rl_repo")

import numpy as np

import concourse.bacc as bacc
import concourse.tile as tile
from concourse import mybir
from concourse.bass_utils import run_bass_kernel_spmd

N1, N2, D, B = 4096, 8192, 128, 1024  # emb1 rows, emb2 rows, dim, num_bits
G1, G2 = 2, 4
M1, M2 = N1 // G1, N2 // G2  # 2048 x 2048 output block per core
KC = B // 128  # 8 bit-chunks of 128
RW = 512  # projection row-chunk width
NW = 512  # main matmul psum tile width

_BUILD_CACHE = {}


def _build(scale: float):
    if scale in _BUILD_CACHE:
        return _BUILD_CACHE[scale]
    nc = bacc.Bacc("TRN2", target_bir_lowering=False, debug=False)
    f32 = mybir.dt.float32
    f32r = mybir.dt.float32r
    bf16 = mybir.dt.bfloat16
    fp8 = mybir.dt.float8e4
    A = mybir.AluOpType
    AF = mybir.ActivationFunctionType

    e1 = nc.declare_dram_parameter("e1", [D, M1], f32r, isOutput=False)
    e2 = nc.declare_dram_parameter("e2", [D, M2], f32r, isOutput=False)
    rr = nc.declare_dram_parameter("rr", [D, B], f32r, isOutput=False)
    out = nc.declare_dram_parameter("out", [M1, M2], bf16, isOutput=True)

    with tile.TileContext(nc) as tc:
        with (
            tc.tile_pool(name="const", bufs=1) as const_pool,
            tc.tile_pool(name="outs", bufs=4) as out_pool,
            tc.tile_pool(name="pproj", bufs=2, space="PSUM") as pp,
            tc.tile_pool(name="pmain", bufs=2, space="PSUM") as mp,
        ):
            r_sb = const_pool.tile([D, B], f32r)
            e1_sb = const_pool.tile([D, M1], f32r)
            e2_sb = const_pool.tile([D, M2], f32r)
            ut = const_pool.tile([128, KC, M1], fp8)
            vt = const_pool.tile([128, KC, M2], fp8)

            # Warm-up memset first so the PE ramp can begin immediately.
            warm = const_pool.tile([128, RW], bf16)
            nc.gpsimd.memset(warm[:], 0.0)

            # Input DMAs, ordered for earliest first-tile availability.
            # scalar (ACT, idle until its first Sign) feeds r-head/e1 via
            # HWDGE; sync feeds e2/r-tail.
            # DMA issue-to-land latency is ~4.8us fixed, so the gating
            # transfers go first on each queue with no extra splitting.
            # The first tile (u j0 c0) needs BOTH r-head and e1j0: put
            # them first on DIFFERENT queues so their ~4.8us latencies
            # run concurrently.
            nc.sync.dma_start(r_sb[:, 0:512], rr[:, 0:512])
            nc.scalar.dma_start(e1_sb[:, 0:RW], e1[:, 0:RW])
            nc.sync.dma_start(e2_sb[:, 0:RW], e2[:, 0:RW])
            nc.scalar.dma_start(e1_sb[:, RW : 2 * RW], e1[:, RW : 2 * RW])
            nc.sync.dma_start(r_sb[:, 512:], rr[:, 512:])
            for j in (1, 2, 3):
                sl = slice(j * RW, (j + 1) * RW)
                nc.sync.dma_start(e2_sb[:, sl], e2[:, sl])
                if j > 1:
                    nc.scalar.dma_start(e1_sb[:, sl], e1[:, sl])

            # HAM warm-up: the PE clock ramps to 2.4GHz only after ~3.5us
            # of sustained activity (trace: proj ran at 1.2GHz until
            # t~20us with only 8 warmups + a gap). Burn enough dummy
            # matmuls to (a) cover the >=3.5us busy window and (b) keep
            # the PE busy until the first input DMAs land (~12us), so
            # projection starts on a warm clock with no PE idle gap.
            for _ in range(12):
                wps = pp.tile([128, 2, RW], f32, name="pjtile", tag="pj")
                nc.tensor.matmul(
                    wps[:, 0, :], warm[:, 0:128], warm[:], start=True, stop=True
                )

            # Engine assignment per (tensor, j-block): a code block's
            # magnitude only needs to be uniform within its own j-block,
            # because each main psum bank reads exactly one u j-block and
            # one v j-block — the per-half Sin scale absorbs the product
            # of magnitudes. ACT Sign gives +/-1, DVE is_gt/sub +/-0.5.
            # u j0/j1 go to ACT (needed early, before ACT has Sin work);
            # everything else goes to DVE, which is otherwise idle during
            # the main phase — this keeps ACT free to stream Sins with no
            # binarize bursts delaying the psum recycle.
            def u_on_act(j):
                return j < 2

            def proj_group(is_u, j, a):
                # Two 128-bit chunks (c=2a, 2a+1) into one 2-bank psum
                # tile, drained by a single binarize instruction: ACT has
                # ~494ns fixed cost per instruction, so 2-bank groups cut
                # the binarize engine time from ~22us to ~16us per engine
                # (the projection phase is binarize-bound, not PE-bound).
                src = e1_sb if is_u else e2_sb
                dst = ut if is_u else vt
                sl = slice(j * RW, (j + 1) * RW)
                ps = pp.tile([128, 2, RW], f32, name="pjtile", tag="pj")
                for i in range(2):
                    cs = slice((2 * a + i) * 128, (2 * a + i + 1) * 128)
                    nc.tensor.matmul(
                        ps[:, i, :], r_sb[:, cs], src[:, sl], start=True, stop=True
                    )
                if is_u and u_on_act(j):
                    nc.scalar.activation(dst[:, 2 * a : 2 * a + 2, sl], ps[:], AF.Sign)
                else:
                    nc.vector.tensor_scalar(
                        dst[:, 2 * a : 2 * a + 2, sl], ps[:], 0.0, 0.5, A.is_gt, A.subtract
                    )

            def vert_half(a, pair, b, dmaq, fine=False):
                # "Vertical half": two adjacent 128-row blocks x one 512
                # column quarter, one psum bank per row block. The second
                # bank's matmuls only need the same (u j, v j) codes, so a
                # vert-half depends on exactly one u j-block and one v
                # j-block — main work can start as soon as u j0 + v j0
                # codes exist.
                ps = mp.tile([128, 2, NW], f32, name="pmtile", tag="pm")
                ot = out_pool.tile([128, 2 * NW], bf16)
                ns = slice(b * NW, (b + 1) * NW)
                # psum = u_mag * v_mag * (s_u . s_v); Sin(scale * psum)
                # needs scale = (pi/2d) / (u_mag * v_mag). v_mag = 0.5
                # always; u_mag = 1 (ACT) for j<2 else 0.5 (DVE).
                sc = scale if u_on_act(a) else 2.0 * scale
                for mi in range(2):
                    m = 4 * a + 2 * pair + mi
                    ms = slice(m * 128, (m + 1) * 128)
                    for s in range(KC // 2):
                        nc.tensor.matmul(
                            ps[:, mi, :],
                            ut[:, 2 * s : 2 * s + 2, ms],
                            vt[:, 2 * s : 2 * s + 2, ns],
                            start=(s == 0),
                            stop=(s == KC // 2 - 1),
                            perf_mode=mybir.MatmulPerfMode.DoubleRow,
                        )
                    if fine:
                        # last vert-halves: per-bank Sin/DMA so bank0's
                        # drain overlaps bank1's matmuls
                        nc.scalar.activation(
                            ot[:, mi * NW : (mi + 1) * NW], ps[:, mi, :], AF.Sin, scale=sc
                        )
                        dmaq.dma_start(out[ms, ns], ot[:, mi * NW : (mi + 1) * NW])
                if not fine:
                    nc.scalar.activation(ot[:], ps[:], AF.Sin, scale=sc)
                    for mi in range(2):
                        m = 4 * a + 2 * pair + mi
                        ms = slice(m * 128, (m + 1) * 128)
                        dmaq.dma_start(out[ms, ns], ot[:, mi * NW : (mi + 1) * NW])

            # Upfront projection: only what the first blocks need — u j0,
            # v j0, v j1 (12 groups). ABAB interleave starts both
            # binarize engines as early as possible; the first vert-half
            # only waits for u j0 + v j0 (~6us of binarize).
            UP = [
                (True, 0, 0), (False, 0, 0), (True, 0, 1), (False, 0, 1),
                (True, 0, 2), (False, 0, 2), (True, 0, 3), (False, 0, 3),
                (False, 1, 0), (False, 1, 1), (False, 1, 2), (False, 1, 3),
            ]
            for g in UP:
                proj_group(*g)

            # Remaining 20 groups (u j1 first — it is needed by block
            # (1,1) — then u/v j2, j3) spread two per vert-half through
            # the early main stream; the binarize engines drain them with
            # slack before first use.
            SPREAD = [(True, 1, a) for a in range(KC // 2)]
            for j in (2, 3):
                for a in range(KC // 2):
                    SPREAD.append((True, j, a))
                    SPREAD.append((False, j, a))

            # Block order (a = u j-block, b = v j-block) grows the set of
            # needed code blocks slowly so binarize is never on the
            # critical path.
            BLOCKS = [
                (0, 0), (0, 1), (1, 1), (1, 0),
                (2, 0), (2, 1), (0, 2), (1, 2),
                (2, 2), (3, 0), (3, 1), (3, 2),
                (0, 3), (1, 3), (2, 3), (3, 3),
            ]
            nvh = 2 * len(BLOCKS)
            vhi = 0
            si = 0
            for (a, b) in BLOCKS:
                for pair in range(2):
                    for _ in range(2):
                        if si < len(SPREAD):
                            proj_group(*SPREAD[si])
                            si += 1
                    # Alternate output DMA queues, but route the tail to
                    # sync (HWDGE) so the gpsimd SWDGE ring drains early.
                    dmaq = nc.sync if (vhi % 2 == 0 or vhi >= nvh - 6) else nc.gpsimd
                    vert_half(a, pair, b, dmaq, fine=(vhi >= nvh - 2))
                    vhi += 1

    nc.compile()
    _BUILD_CACHE[scale] = nc
    return nc


def _r22(a):
    """Round fp32 to 13 mantissa bits (nearest-even) — the PE's fp32r
    path truncates operands to ~FP22, so pre-rounding on the host turns
    that truncation into a no-op and halves the effective input error."""
    u = a.view(np.uint32)
    lsb = (u >> np.uint32(10)) & np.uint32(1)
    return ((u + np.uint32(0x1FF) + lsb) & np.uint32(0xFFFFFC00)).view(np.float32)


def _in_maps(emb1, emb2, r):
    rt = _r22(np.ascontiguousarray(r.T))
    e1t = _r22(np.ascontiguousarray(emb1.T))
    e2t = _r22(np.ascontiguousarray(emb2.T))
    maps = []
    for k in range(8):
        a, b = k // G2, k % G2
        maps.append(
            {
                "e1": np.ascontiguousarray(e1t[:, a * M1 : (a + 1) * M1]),
                "e2": np.ascontiguousarray(e2t[:, b * M2 : (b + 1) * M2]),
                "rr": rt,
            }
        )
    return maps


def _install_profile_hook():
    """The agent image's antenv lacks axon_hooks; synthesize it so
    run_bass_kernel_spmd(trace=True) can reach the NTFF profiler."""
    import types

    if "antenv.axon_hooks" in sys.modules:
        return
    try:
        from trn_agent_boot.trn_boot import _ntff_profile_via_ctypes

        hook = _ntff_profile_via_ctypes("/opt/axon/libaxon_pjrt.so")
        mod = types.ModuleType("antenv.axon_hooks")
        mod.get_axon_ntff_profile_hook = lambda: hook
        sys.modules["antenv.axon_hooks"] = mod

        from concourse import bass_utils as _bu

        _orig_upload = _bu.upload_artifacts

        def _safe_upload(tmpdir):
            try:
                return _orig_upload(tmpdir)
            except Exception as e:  # no bucket access in this container
                return f"upload-skipped: {e}"

        _bu.upload_artifacts = _safe_upload
    except Exception:
        pass


def kernel(emb1, emb2, r, pi, _trace=False, _tmpdir=None):
    emb1 = np.asarray(emb1, dtype=np.float32)
    emb2 = np.asarray(emb2, dtype=np.float32)
    r = np.asarray(r, dtype=np.float32)
    # u codes +/-1, v codes +/-0.5: psum P = 0.5 * s_u.s_v, out = sin((pi/B)*P)
    scale = float(np.asarray(pi).reshape(-1)[0]) / B

    nc = _build(scale)
    if _trace:
        _install_profile_hook()
    try:
        res = run_bass_kernel_spmd(
            nc, _in_maps(emb1, emb2, r), list(range(8)), trace=_trace, tmpdir=_tmpdir
        )
    except ModuleNotFoundError:
        res = run_bass_kernel_spmd(nc, _in_maps(emb1, emb2, r), list(range(8)))

    full = np.empty((N1, N2), dtype=np.float32)
    for k in range(8):
        a, b = k // G2, k % G2
        full[a * M1 : (a + 1) * M1, b * M2 : (b + 1) * M2] = np.asarray(
            res.results[k]["out"]
        ).astype(np.float32)
    if _trace:
        kernel._last_exec_time_ns = res.exec_time_ns
    return full



# revision 11
# speedup vs baseline: 1.0390x; 1.0390x over previous
"""LSH cosine-of-Hamming retrieval kernel for 8 trn2 NeuronCores.

Math: reference computes cos((pi/d) * hamming(u, v)) for binary LSH codes
u = (emb1 @ r.T > 0), v = (emb2 @ r.T > 0), d = 1024 bits.
With +/-1 sign codes s_u = 2u-1, s_v = 2v-1:
    hamming = (d - s_u . s_v) / 2
    cos((pi/d) * hamming) = sin((pi/2d) * s_u.s_v)
u codes are stored as +/-1 fp8 (ACT Sign), v codes as +/-0.5 fp8 (DVE
is_gt/subtract), so psum P = 0.5 * s_u.s_v and out = sin((pi/d) * P).

Projection runs in ONE fp32r pass (the PE truncates operands to ~FP22
and products to similar precision; inputs are pre-rounded to 13
mantissa bits on the host so the input truncation is a no-op, leaving
~6e-5 sign-flip rate -> 9.6e-3 rel err, inside the 2e-2 budget). This
replaces the 3-pass bf16 hi/lo split projection: 64 instead of 192
projection matmuls (fp32r streams ~1.7 cycles/row vs bf16's 1.0, so
projection PE time drops 41us -> ~24us).

The binarize of the projection psum (4M fp32 values) is the side
constraint: GPSIMD cannot read PSUM and DVE 2x perf modes need 16-bit
or SBUF sources, so only DVE (~0.69us/bank) and ACT (~0.69us/bank) can
drain codes. Code-magnitude uniformity within each tensor forces u
entirely on ACT (Sign, +/-1) and v entirely on DVE (+/-0.5); both run
saturated through the projection phase and their tails overlap the
main-matmul phase (early halves only need v j0/j1 + u j0 codes).

Main phase: fp8 DoubleRow half-blocks (128 rows x 1024 cols), one
2-bank psum pair per half, Sin on ACT, output DMAs alternating
sync/gpsimd queues. The last two halves use finer Sin/DMA quanta to
shorten the drain.

NOTE: do NOT dedupe back-to-back identical LDWEIGHTS here - on this
runtime the PE's reorder window executes pulled-ahead weight loads
while early matmuls are parked on unsatisfied code semaphores, and the
second matmul of a pair then runs with clobbered weights (reproducible
NaN quadrants in the first scheduled halves).

Sharding (2x4 grid over 8 cores): core k computes the [2048, 2048]
output block for emb1 rows [(k//4)*2048...] x emb2 rows [(k%4)*2048...];
r is replicated (collectives cost ~60us fixed here - not worth it).
Host prep is layout-only: transpose + fp22 pre-round.
"""

import sys

sys.path.insert(0, "/opt/trn_rl_repo")

import numpy as np

import concourse.bacc as bacc
import concourse.tile as tile
from concourse import mybir
from concourse.bass_utils import run_bass_kernel_spmd

N1, N2, D, B = 4096, 8192, 128, 1024  # emb1 rows, emb2 rows, dim, num_bits
G1, G2 = 2, 4
M1, M2 = N1 // G1, N2 // G2  # 2048 x 2048 output block per core
KC = B // 128  # 8 bit-chunks of 128
RW = 512  # projection row-chunk width
NW = 512  # main matmul psum tile width

_BUILD_CACHE = {}


def _build(scale: float):
    if scale in _BUILD_CACHE:
        return _BUILD_CACHE[scale]
    nc = bacc.Bacc("TRN2", target_bir_lowering=False, debug=False)
    f32 = mybir.dt.float32
    f32r = mybir.dt.float32r
    bf16 = mybir.dt.bfloat16
    fp8 = mybir.dt.float8e4
    A = mybir.AluOpType
    AF = mybir.ActivationFunctionType

    e1 = nc.declare_dram_parameter("e1", [D, M1], f32r, isOutput=False)
    e2 = nc.declare_dram_parameter("e2", [D, M2], f32r, isOutput=False)
    rr = nc.declare_dram_parameter("rr", [D, B], f32r, isOutput=False)
    out = nc.declare_dram_parameter("out", [M1, M2], bf16, isOutput=True)

    with tile.TileContext(nc) as tc:
        with (
            tc.tile_pool(name="const", bufs=1) as const_pool,
            tc.tile_pool(name="outs", bufs=4) as out_pool,
            tc.tile_pool(name="pproj", bufs=2, space="PSUM") as pp,
            tc.tile_pool(name="pmain", bufs=2, space="PSUM") as mp,
        ):
            r_sb = const_pool.tile([D, B], f32r)
            e1_sb = const_pool.tile([D, M1], f32r)
            e2_sb = const_pool.tile([D, M2], f32r)
            ut = const_pool.tile([128, KC, M1], fp8)
            vt = const_pool.tile([128, KC, M2], fp8)

            # Warm-up memset first so the PE ramp can begin immediately.
            warm = const_pool.tile([128, RW], bf16)
            nc.gpsimd.memset(warm[:], 0.0)

            # Input DMAs, ordered for earliest first-tile availability.
            # scalar (ACT, idle until its first Sign) feeds r-head/e1 via
            # HWDGE; sync feeds e2/r-tail.
            # DMA issue-to-land latency is ~4.8us fixed, so the gating
            # transfers go first on each queue with no extra splitting.
            # The first tile (u j0 c0) needs BOTH r-head and e1j0: put
            # them first on DIFFERENT queues so their ~4.8us latencies
            # run concurrently.
            nc.sync.dma_start(r_sb[:, 0:512], rr[:, 0:512])
            nc.scalar.dma_start(e1_sb[:, 0:RW], e1[:, 0:RW])
            nc.sync.dma_start(r_sb[:, 512:], rr[:, 512:])
            for j in range(4):
                sl = slice(j * RW, (j + 1) * RW)
                nc.sync.dma_start(e2_sb[:, sl], e2[:, sl])
                if j > 0:
                    nc.scalar.dma_start(e1_sb[:, sl], e1[:, sl])

            # HAM warm-up: the PE clock ramps to 2.4GHz only after ~3.5us
            # of sustained activity (trace: proj ran at 1.2GHz until
            # t~20us with only 8 warmups + a gap). Burn enough dummy
            # matmuls to (a) cover the >=3.5us busy window and (b) keep
            # the PE busy until the first input DMAs land (~12us), so
            # projection starts on a warm clock with no PE idle gap.
            for _ in range(12):
                wps = pp.tile([128, 2, RW], f32, name="pjtile", tag="pj")
                nc.tensor.matmul(
                    wps[:, 0, :], warm[:, 0:128], warm[:], start=True, stop=True
                )

            # Engine assignment per (tensor, j-block): a code block's
            # magnitude only needs to be uniform within its own j-block,
            # because each main psum bank reads exactly one u j-block and
            # one v j-block — the per-half Sin scale absorbs the product
            # of magnitudes. ACT Sign gives +/-1, DVE is_gt/sub +/-0.5.
            # u j0/j1 go to ACT (needed early, before ACT has Sin work);
            # everything else goes to DVE, which is otherwise idle during
            # the main phase — this keeps ACT free to stream Sins with no
            # binarize bursts delaying the psum recycle.
            def u_on_act(j):
                return j == 0

            def proj_group(is_u, j, a):
                # Two 128-bit chunks (c=2a, 2a+1) into one 2-bank psum
                # tile, drained by a single binarize instruction: ACT has
                # ~494ns fixed cost per instruction, so 2-bank groups cut
                # the binarize engine time from ~22us to ~16us per engine
                # (the projection phase is binarize-bound, not PE-bound).
                src = e1_sb if is_u else e2_sb
                dst = ut if is_u else vt
                sl = slice(j * RW, (j + 1) * RW)
                ps = pp.tile([128, 2, RW], f32, name="pjtile", tag="pj")
                for i in range(2):
                    cs = slice((2 * a + i) * 128, (2 * a + i + 1) * 128)
                    nc.tensor.matmul(
                        ps[:, i, :], r_sb[:, cs], src[:, sl], start=True, stop=True
                    )
                if is_u and u_on_act(j):
                    nc.scalar.activation(dst[:, 2 * a : 2 * a + 2, sl], ps[:], AF.Sign)
                else:
                    nc.vector.tensor_scalar(
                        dst[:, 2 * a : 2 * a + 2, sl], ps[:], 0.0, 0.5, A.is_gt, A.subtract
                    )

            def vert_half(a, pair, b, dmaq, fine=False):
                # "Vertical half": two adjacent 128-row blocks x one 512
                # column quarter, one psum bank per row block. The second
                # bank's matmuls only need the same (u j, v j) codes, so a
                # vert-half depends on exactly one u j-block and one v
                # j-block — main work can start as soon as u j0 + v j0
                # codes exist.
                ps = mp.tile([128, 2, NW], f32, name="pmtile", tag="pm")
                ot = out_pool.tile([128, 2 * NW], bf16)
                ns = slice(b * NW, (b + 1) * NW)
                # psum = u_mag * v_mag * (s_u . s_v); Sin(scale * psum)
                # needs scale = (pi/2d) / (u_mag * v_mag). v_mag = 0.5
                # always; u_mag = 1 (ACT) for j<2 else 0.5 (DVE).
                sc = scale if u_on_act(a) else 2.0 * scale  # u j0 is +/-1, rest +/-0.5
                for mi in range(2):
                    m = 4 * a + 2 * pair + mi
                    ms = slice(m * 128, (m + 1) * 128)
                    for s in range(KC // 2):
                        nc.tensor.matmul(
                            ps[:, mi, :],
                            ut[:, 2 * s : 2 * s + 2, ms],
                            vt[:, 2 * s : 2 * s + 2, ns],
                            start=(s == 0),
                            stop=(s == KC // 2 - 1),
                            perf_mode=mybir.MatmulPerfMode.DoubleRow,
                        )
                    if fine:
                        # last vert-halves: per-bank Sin/DMA so bank0's
                        # drain overlaps bank1's matmuls
                        nc.scalar.activation(
                            ot[:, mi * NW : (mi + 1) * NW], ps[:, mi, :], AF.Sin, scale=sc
                        )
                        dmaq.dma_start(out[ms, ns], ot[:, mi * NW : (mi + 1) * NW])
                if not fine:
                    nc.scalar.activation(ot[:], ps[:], AF.Sin, scale=sc)
                    for mi in range(2):
                        m = 4 * a + 2 * pair + mi
                        ms = slice(m * 128, (m + 1) * 128)
                        dmaq.dma_start(out[ms, ns], ot[:, mi * NW : (mi + 1) * NW])

            # Upfront projection: only what the first blocks need — u j0,
            # v j0, v j1 (12 groups). ABAB interleave starts both
            # binarize engines as early as possible; the first vert-half
            # only waits for u j0 + v j0 (~6us of binarize).
            # u groups lead while e2 chunks are still landing (sync queue
            # carries r then e2; scalar carries e1).
            UP = [
                (True, 0, 0), (True, 0, 1), (False, 0, 0), (True, 0, 2),
                (False, 0, 1), (True, 0, 3), (False, 0, 2), (False, 0, 3),
                (False, 1, 0), (False, 1, 1), (False, 1, 2), (False, 1, 3),
            ]
            for g in UP:
                proj_group(*g)

            # Remaining 20 groups (u j1 first — needed by block (1,1) —
            # then j2, j3) spread one per vert-half through the main
            # stream; DVE (their binarize engine) drains them with slack
            # before first use.
            SPREAD = (
                [(True, 1, a) for a in range(KC // 2)]
                + [(True, 2, a) for a in range(KC // 2)]
                + [(False, 2, a) for a in range(KC // 2)]
                + [(True, 3, a) for a in range(KC // 2)]
                + [(False, 3, a) for a in range(KC // 2)]
            )

            # Block order (a = u j-block, b = v j-block) grows the set of
            # needed code blocks slowly so binarize is never on the
            # critical path.
            BLOCKS = [
                (0, 0), (0, 1), (1, 1), (1, 0),
                (2, 0), (2, 1), (0, 2), (1, 2),
                (2, 2), (3, 0), (3, 1), (3, 2),
                (0, 3), (1, 3), (2, 3), (3, 3),
            ]
            nvh = 2 * len(BLOCKS)
            vhi = 0
            si = 0
            for (a, b) in BLOCKS:
                for pair in range(2):
                    if si < len(SPREAD):
                        proj_group(*SPREAD[si])
                        si += 1
                    # Alternate output DMA queues, but route the tail to
                    # sync (HWDGE) so the gpsimd SWDGE ring drains early.
                    dmaq = nc.sync if (vhi % 2 == 0 or vhi >= nvh - 6) else nc.gpsimd
                    vert_half(a, pair, b, dmaq, fine=(vhi >= nvh - 2))
                    vhi += 1

    nc.compile()
    _BUILD_CACHE[scale] = nc
    return nc


def _r22(a):
    """Round fp32 to 13 mantissa bits (nearest-even) — the PE's fp32r
    path truncates operands to ~FP22, so pre-rounding on the host turns
    that truncation into a no-op and halves the effective input error."""
    u = a.view(np.uint32)
    lsb = (u >> np.uint32(10)) & np.uint32(1)
    return ((u + np.uint32(0x1FF) + lsb) & np.uint32(0xFFFFFC00)).view(np.float32)


def _in_maps(emb1, emb2, r):
    rt = _r22(np.ascontiguousarray(r.T))
    e1t = _r22(np.ascontiguousarray(emb1.T))
    e2t = _r22(np.ascontiguousarray(emb2.T))
    maps = []
    for k in range(8):
        a, b = k // G2, k % G2
        maps.append(
            {
                "e1": np.ascontiguousarray(e1t[:, a * M1 : (a + 1) * M1]),
                "e2": np.ascontiguousarray(e2t[:, b * M2 : (b + 1) * M2]),
                "rr": rt,
            }
        )
    return maps


def _install_profile_hook():
    """The agent image's antenv lacks axon_hooks; synthesize it so
    run_bass_kernel_spmd(trace=True) can reach the NTFF profiler."""
    import types

    if "antenv.axon_hooks" in sys.modules:
        return
    try:
        from trn_agent_boot.trn_boot import _ntff_profile_via_ctypes

        hook = _ntff_profile_via_ctypes("/opt/axon/libaxon_pjrt.so")
        mod = types.ModuleType("antenv.axon_hooks")
        mod.get_axon_ntff_profile_hook = lambda: hook
        sys.modules["antenv.axon_hooks"] = mod

        from concourse import bass_utils as _bu

        _orig_upload = _bu.upload_artifacts

        def _safe_upload(tmpdir):
            try:
                return _orig_upload(tmpdir)
            except Exception as e:  # no bucket access in this container
                return f"upload-skipped: {e}"

        _bu.upload_artifacts = _safe_upload
    except Exception:
        pass


def kernel(emb1, emb2, r, pi, _trace=False, _tmpdir=None):
    emb1 = np.asarray(emb1, dtype=np.float32)
    emb2 = np.asarray(emb2, dtype=np.float32)
    r = np.asarray(r, dtype=np.float32)
    # u codes +/-1, v codes +/-0.5: psum P = 0.5 * s_u.s_v, out = sin((pi/B)*P)
    scale = float(np.asarray(pi).reshape(-1)[0]) / B

    nc = _build(scale)
    if _trace:
        _install_profile_hook()
    try:
        res = run_bass_kernel_spmd(
            nc, _in_maps(emb1, emb2, r), list(range(8)), trace=_trace, tmpdir=_tmpdir
        )
    except ModuleNotFoundError:
        res = run_bass_kernel_spmd(nc, _in_maps(emb1, emb2, r), list(range(8)))

    full = np.empty((N1, N2), dtype=np.float32)
    for k in range(8):
        a, b = k // G2, k % G2
        full[a * M1 : (a + 1) * M1, b * M2 : (b + 1) * M2] = np.asarray(
            res.results[k]["out"]
        ).astype(np.float32)
    if _trace:
        kernel._last_exec_time_ns = res.exec_time_ns
    return full

